# revision 78
# baseline (speedup 1.0000x reference)
"""Trainium2 Bass kernel for the attention-MLP problem.

Reference computation (S=32768, H=1024):
    cat    = [broadcast(hidden, (S, 2H)) | encoder_output]   # [S, 3H]
    energy = tanh(cat @ attn_w.T + attn_b)                   # [S, H]
    logits = (energy @ v_w.T).squeeze()                      # [S]
    out    = softmax(logits)                                 # [S]

Split: c0 = hidden @ W1T + b (one row) and pre = enc @ W2T + c0, with
W1T = attn_w[:, :2H].T, W2T = attn_w[:, 2H:].T.

Sharding: seq split across 8 cores (4096 rows each), weights replicated.
c0 is computed fully on every core (w1 is only 4.2MB bf16 of overlapped
DMA) — no AllReduce, no cross-core sync at the head. Softmax
normalization: exp (no max needed, |logits| <= ||v_w||_1 ~ 26), one
AllGather of the 8 per-core partial sums at the tail.

v2 layout/schedule (all matmul data bf16):
  - OUT^T per-core: psum tiles hold energy^T [j, s] blocks [128, 512].
  - Groups j0,j1 run seq-major over panel 0 (one 8-matmul k-chain per
    512-col block, paced by the enc DMA stream) so the PE starts on the
    first arriving tile; the c0 row matmuls slot into the chain gaps as
    the two w1 halves land. All other groups run k-major over 4-block
    panels (two panels double-buffer in the 8 psum banks).
  - v-dot moved off the PE: DVE scalar_tensor_tensor accumulates
    acc += tanh_tile * vw[j] per group; a single ones-vector matmul per
    seq block reduces acc over partitions (panel 0's reduce+exp slots
    into panel 1's first k-sweeps).
  - c0 psum uses one bank via tile_position rows 0/32; the [1,1024] row
    transposes to [128,8] via a DRAM roundtrip; bias added in place.
  - DMA queues: SP = panel-0 enc; ACT HWDGE = weight slabs; Pool = tiny
    constants + the c0 roundtrip, with the late bulk (w2_3..7, enc4..7)
    queued BEHIND it so the small transfers are never starved.
  (An rdma-based hand-rolled all-gather (ag="rdma") simulates ~12us
  faster but the gpsimd extended-ISA ops don't encode on this walrus
  build, so the collective AllGather stays the default.)
"""

import numpy as np

import concourse.bass as bass
import concourse.mybir as mybir
import concourse.tile as tile
from concourse import library_config
from concourse.bass_utils import run_bass_kernel_spmd

H = 1024
S = 32768
NCORES = 8
SL = S // NCORES          # 4096 rows per core
SB = 512                  # seq block (columns of the psum tiles)
NSB = SL // SB            # 8 seq blocks per core
KC = H // 128             # 8 contraction chunks
JC = H // 128             # 8 output-row chunks
NKC = 16 // NCORES        # local w1 chunks (c0 contraction sharded)

F32 = mybir.dt.float32
F32R = mybir.dt.float32r
BF16 = mybir.dt.bfloat16

AF = mybir.ActivationFunctionType
ALU = mybir.AluOpType


# ---------------------------------------------------------------------------
# Workaround for this walrus build: instructions only accept a single
# sync-wait command, but Tile can attach several. Hoist the extra waits
# onto NOPs inserted just before the instruction on the same engine
# (engines execute their stream in order, so semantics are preserved).
def _split_multi_waits(nc):
    end_bb = nc.cur_bb.bb
    for bb in nc.m.functions[0].blocks:
        insts = list(bb.instructions)
        out = []
        changed = False
        for inst in insts:
            si = inst.sync_info
            waits = list(si.on_wait) if si and si.on_wait else []
            if len(waits) > 1:
                changed = True
                for w in waits[:-1]:
                    nop = nc.engines[inst.engine].nop(nofuse=True).ins
                    end_bb.instructions.remove(nop)
                    nop.sync_info = mybir.SyncInfo(on_wait=[w], on_update=[])
                    out.append(nop)
                si.on_wait = waits[-1:]
            out.append(inst)
        if changed:
            bb.instructions = out
# ---------------------------------------------------------------------------


def build(repeat: int = 1, mode: str = "full", single_core: bool = False,
          ag: str = "cc"):
    """Build the per-core Bass module. `repeat` wraps the main compute
    (j-group loop incl. enc DMA + exp) in a For_i loop for marginal-cost
    benchmarking; c0 + collectives + final scale stay outside.
    mode: full | mm_only | dma_only (perf experiments)."""
    mm_only = mode in ("mm_only", "dma_only", "mm_kall", "mm_sall")
    nc = bass.Bass("TRN2", target_bir_lowering=False, debug=False,
                   num_devices=1 if single_core else NCORES)

    NKF = 16   # full c0 contraction chunks (c0 replicated, no AllReduce)
    encT = nc.dram_tensor("encT", [H, SL], BF16, kind="ExternalInput").ap()
    w2t = nc.dram_tensor("w2t", [H, H], BF16, kind="ExternalInput").ap()
    w1s = nc.dram_tensor("w1s", [NKF * 128, H], BF16,
                         kind="ExternalInput").ap()
    hidT = nc.dram_tensor("hidT", [128, NKF], BF16,
                          kind="ExternalInput").ap()
    bT = nc.dram_tensor("bT", [128, JC], F32, kind="ExternalInput").ap()
    vwc = nc.dram_tensor("vwc", [128, JC], F32, kind="ExternalInput").ap()
    onesd = nc.dram_tensor("onesd", [128, 1], F32R,
                           kind="ExternalInput").ap()
    onesr = nc.dram_tensor("onesr", [1, 128], F32,
                           kind="ExternalInput").ap()
    out = nc.dram_tensor("out", [1, SL], F32, kind="ExternalOutput").ap()

    encT_v = encT.rearrange("(k p) s -> p k s", p=128)   # [128, 8, 4096]
    w2t_v = w2t.rearrange("(k p) j -> p k j", p=128)     # [128, 8, 1024]
    w1s_v = w1s.rearrange("(k p) j -> p k j", p=128)     # [128, 16, 1024]

    with tile.TileContext(nc) as tc:
        with (
            tc.tile_pool(name="const", bufs=1) as const_pool,
            tc.tile_pool(name="enc", bufs=9) as enc_pool,
            tc.tile_pool(name="tanh", bufs=4) as tanh_pool,
            tc.tile_pool(name="sm", bufs=1) as sm_pool,
            tc.tile_pool(name="pse", bufs=8, space="PSUM") as pse_pool,
            tc.tile_pool(name="dram", bufs=1, space="DRAM") as dram_pool,
        ):
            # Queue assignment (wire order ~= per-queue dispatch-completion
            # order, ~0.6-1.3us per dispatch): SP carries the panel-0 enc
            # stream; the ACT HWDGE queue carries the early weight slabs
            # (w2_0, w1 halves, w2_1, w2_2 interleaved); Pool (slow SWDGE)
            # carries tiny constants, the c0 transpose DMA, and — queued
            # BEHIND it so they can't starve it — the late bulk (w2_3..7,
            # enc4..7).
            w2_tiles = [const_pool.tile([128, KC, 128], BF16, name=f"w2_{j}")
                        for j in range(JC)]
            w1_a = const_pool.tile([128, NKF // 2, H], BF16)
            w1_b = const_pool.tile([128, NKF // 2, H], BF16)
            nc.scalar.dma_start(w2_tiles[0][:], w2t_v[:, :, 0:128])
            nc.scalar.dma_start(w1_a[:], w1s_v[:, :NKF // 2, :])
            nc.scalar.dma_start(w2_tiles[1][:], w2t_v[:, :, 128:256])
            nc.scalar.dma_start(w1_b[:], w1s_v[:, NKF // 2:, :])
            nc.scalar.dma_start(w2_tiles[2][:], w2t_v[:, :, 256:384])
            hid_sb = const_pool.tile([128, NKF], BF16)
            nc.gpsimd.dma_start(hid_sb[:], hidT[:])
            vw_sb = const_pool.tile([128, JC], F32)
            nc.gpsimd.dma_start(vw_sb[:], vwc[:])
            bT_sb = const_pool.tile([128, JC], F32)
            nc.gpsimd.dma_start(bT_sb[:], bT[:])
            ones_sb = const_pool.tile([128, 1], F32R)
            nc.gpsimd.dma_start(ones_sb[:], onesd[:])
            onesr_sb = const_pool.tile([1, 128], F32)
            nc.gpsimd.dma_start(onesr_sb[:], onesr[:])
            if repeat != 1:
                for j in range(3, JC):
                    nc.scalar.dma_start(w2_tiles[j][:],
                                        w2t_v[:, :, j * 128:(j + 1) * 128])

            c0_sb = const_pool.tile([128, JC], F32)
            acc = [sm_pool.tile([128, SB], F32R, name=f"acc{sb}")
                   for sb in range(NSB)]
            exps = sm_pool.tile([1, SL], F32)
            sums = sm_pool.tile([1, NSB], F32)
            if not single_core and ag == "rdma":
                zsg = sm_pool.tile([128, NCORES, NSB], F32)
                ag_rsem = nc.alloc_semaphore("ag_rsem")
                ag_lsem = nc.alloc_semaphore("ag_lsem")
                # partition_broadcast + remote-DMA desc-gen are gpsimd ucode
                # ops from the 'proxy' library; load it once at kernel start
                nc.gpsimd.load_library(library_config.proxy)

            # --- c0 = hidden @ W1T + b, replicated on every core (no
            # collective). Row layout, both psum halves in one bank via
            # tile_position rows 0/32; transpose to [128, 8] via an
            # SBUF->SBUF DMA; bias added in place. ------------------------
            c0_row = const_pool.tile([1, H], F32)

            def c0_mms(psum_c, kcs, w1_t, koff):
                for kc in kcs:
                    for half in range(2):
                        nc.tensor.matmul(
                            psum_c[32 * half:32 * half + 1, :],
                            hid_sb[:, koff + kc:koff + kc + 1],
                            w1_t[:, kc, half * SB:(half + 1) * SB],
                            tile_position=(0, 32 * half),
                            start=(koff + kc == 0),
                            stop=(koff + kc == NKF - 1),
                            skip_group_check=True,
                        )

            def c0_finish(psum_c):
                for half in range(2):
                    nc.scalar.activation(
                        c0_row[:, half * SB:(half + 1) * SB],
                        psum_c[32 * half:32 * half + 1, :], AF.Identity)
                c0d = dram_pool.tile([1, H], F32)
                nc.gpsimd.dma_start(c0d[:], c0_row[:])
                nc.gpsimd.dma_start(
                    c0_sb[:],
                    c0d[:].rearrange("o (j p) -> (o p) j", p=128))
                nc.vector.scalar_tensor_tensor(
                    c0_sb[:], c0_sb[:], 1.0, bT_sb[:],
                    op0=ALU.bypass, op1=ALU.add)

            def c0_section():
                psum_c = pse_pool.tile([128, SB], F32, tag="ps", bufs=8,
                                       name="psum_c")
                c0_mms(psum_c, range(NKF // 2), w1_a, 0)
                c0_mms(psum_c, range(NKF // 2), w1_b, NKF // 2)
                c0_finish(psum_c)

            def tanh_stt(psum_e, j, sb, first):
                th = tanh_pool.tile([128, SB], F32R)
                nc.scalar.activation(
                    th[:], psum_e[:], AF.Tanh, bias=c0_sb[:, j:j + 1])
                a = acc[sb][:]
                if first:
                    nc.vector.tensor_scalar_mul(a, th[:], vw_sb[:, j:j + 1])
                else:
                    nc.vector.scalar_tensor_tensor(
                        a, th[:], vw_sb[:, j:j + 1], a,
                        op0=ALU.mult, op1=ALU.add,
                    )

            # --- main pipeline -------------------------------------------
            PAN = 4    # seq blocks per k-major panel (2 panels in flight
            #            double-buffer in the 8 psum banks)

            def kmajor_group(j, sbs, enc_ts, interleave=()):
                pes = [pse_pool.tile([128, SB], F32, tag="ps", bufs=8,
                                     name="pe")
                       for _ in sbs]
                interleave = list(interleave)
                for k in range(KC):
                    w = w2_tiles[j][:, k, :]
                    for i, sb in enumerate(sbs):
                        nc.tensor.matmul(
                            pes[i][:], w, enc_ts[sb][:, k, :],
                            start=(k == 0), stop=(k == KC - 1),
                        )
                    if interleave:
                        interleave.pop(0)()
                if not mm_only:
                    for i, sb in enumerate(sbs):
                        tanh_stt(pes[i], j, sb, first=(j == 0))
                for fn in interleave:
                    fn()

            def reduce_exp(sb):
                psum_r = pse_pool.tile([128, SB], F32, tag="ps", bufs=8,
                                       name="pr")
                nc.tensor.matmul(
                    psum_r[0:1, :], ones_sb[:], acc[sb][:],
                    start=True, stop=True,
                )
                nc.scalar.activation(
                    exps[:, sb * SB:(sb + 1) * SB],
                    psum_r[0:1, :], AF.Exp,
                    accum_out=sums[:, sb:sb + 1],
                )

            def main_body(_iv=None, first=False):
                # enc arrives seq-block-major: [128, 8k, 512] per block.
                # Single-shot: panel 1's blocks and w2_3..7 ride the Pool
                # queue BEHIND the c0 transpose DMA (emitted mid-phase-A)
                # so that tiny transfer is never starved by bulk traffic.
                # They aren't needed until ~halfway through the kernel.
                enc_ts = []
                for sb in range(NSB):
                    e = enc_pool.tile([128, KC, SB], BF16, tag="enc",
                                      bufs=9, name="enc")
                    if first and sb >= PAN:
                        enc_ts.append(e)
                        continue
                    nc.sync.dma_start(
                        e[:], encT_v[:, :, sb * SB:(sb + 1) * SB])
                    enc_ts.append(e)

                def late_bulk():
                    for j in range(3, JC):
                        nc.gpsimd.dma_start(
                            w2_tiles[j][:],
                            w2t_v[:, :, j * 128:(j + 1) * 128])
                    for sb in range(PAN, NSB):
                        nc.gpsimd.dma_start(
                            enc_ts[sb][:],
                            encT_v[:, :, sb * SB:(sb + 1) * SB])
                if mode == "dma_only":
                    if first:
                        late_bulk()
                    return
                if first and mm_only:
                    late_bulk()
                if mode == "mm_wide":
                    # probe: 1024-col matmuls spanning two psum banks
                    wide = enc_pool.tile([128, KC, 2 * SB], BF16, tag="wide",
                                         bufs=1, name="wide")
                    nc.sync.dma_start(wide[:], encT_v[:, :, 0:2 * SB])
                    for rep in range(4):
                        for j in range(JC):
                            pw = pse_pool.tile([128, 2 * SB], F32, tag="pw",
                                               bufs=4, name="pw")
                            for k in range(KC):
                                nc.tensor.matmul(
                                    pw[:], w2_tiles[j][:, k, :],
                                    wide[:, k, :],
                                    start=(k == 0), stop=(k == KC - 1),
                                )
                    return
                if mode == "mm_kall":
                    for p in (list(range(PAN)), list(range(PAN, NSB))):
                        for j in range(JC):
                            kmajor_group(j, p, enc_ts)
                    return
                if mode == "mm_sall":
                    for sb in range(NSB):
                        for j in range(JC):
                            psum_e = pse_pool.tile([128, SB], F32, tag="ps",
                                                   bufs=8, name="pe")
                            for k in range(KC):
                                nc.tensor.matmul(
                                    psum_e[:], w2_tiles[j][:, k, :],
                                    enc_ts[sb][:, k, :],
                                    start=(k == 0), stop=(k == KC - 1),
                                )
                    return
                # phase A: groups j0,j1 on panel 0 as seq-major k-chains,
                # paced by the enc DMA stream. In the single-shot path the
                # replicated-c0 matmuls slot into the chain gaps as the two
                # w1 halves land, and the late bulk queues behind the c0
                # transpose DMA.
                # On the single-shot path the first 4 chains' tanh+stt are
                # emitted only after c0_finish: a tanh before c0's writers
                # in trace order would make Tile see a read-before-write on
                # c0_sb instead of a dependency.
                psum_c = None
                pend = []
                if first:
                    psum_c = pse_pool.tile([128, SB], F32, tag="ps", bufs=8,
                                           name="psum_c")
                for sb in range(PAN):
                    for j in range(2):
                        psum_e = pse_pool.tile([128, SB], F32, tag="ps",
                                               bufs=8, name="pe")
                        for k in range(KC):
                            nc.tensor.matmul(
                                psum_e[:], w2_tiles[j][:, k, :],
                                enc_ts[sb][:, k, :],
                                start=(k == 0), stop=(k == KC - 1),
                            )
                        if not mm_only:
                            if first and sb < 2:
                                pend.append((psum_e, j, sb))
                            else:
                                tanh_stt(psum_e, j, sb, first=(j == 0))
                    if first and sb == 0:
                        c0_mms(psum_c, range(NKF // 2), w1_a, 0)
                    if first and sb == 1:
                        c0_mms(psum_c, range(NKF // 2), w1_b, NKF // 2)
                        c0_finish(psum_c)
                        for pe_t, j_, sb_ in pend:
                            tanh_stt(pe_t, j_, sb_, first=(j_ == 0))
                        pend = []
                        late_bulk()
                # phase B: groups j2..j7 k-major on panel 0
                p0 = list(range(PAN))
                p1 = list(range(PAN, NSB))
                for j in range(2, JC):
                    kmajor_group(j, p0, enc_ts)
                # phases C+D: all groups k-major on panel 1; panel 0's
                # logit-reduce + exp slot into phase C's first k-sweeps
                for j in range(JC):
                    il = [lambda sb=sb: reduce_exp(sb) for sb in p0] \
                        if (j == 0 and not mm_only) else ()
                    kmajor_group(j, p1, enc_ts, interleave=il)
                if not mm_only:
                    for sb in p1:
                        reduce_exp(sb)

            if repeat == 1:
                main_body(first=True)
            else:
                if not mm_only:
                    c0_section()
                with tc.For_i(0, repeat, 1,
                              hint_engines=(mybir.EngineType.PE,)) as _i:
                    main_body(_i)

            # --- softmax normalization across cores -----------------------
            if mm_only:
                nc.gpsimd.memset(exps[:], 1.0)
                nc.gpsimd.memset(sums[:], 1.0)
            if single_core:
                zg = sm_pool.tile([1, 1], F32)
                nc.vector.reduce_sum(zg[:], sums[:],
                                     axis=mybir.AxisListType.X)
            elif ag == "rdma":
                # hand-rolled XOR all-gather of the per-core partial sums
                # via direct remote DMA: broadcast k lands on core own^k's
                # slot k, so slot k on core r holds core r^k's row — a
                # bijection, and the sum doesn't care about order. ~3us vs
                # ~15us for the runtime AllGather collective.
                # broadcast the sums row to all 128 partitions with a K=1
                # matmul (the PartitionBroadcast ucode op doesn't encode on
                # this toolchain)
                bsum_ps = pse_pool.tile([128, SB], F32, tag="ps", bufs=8,
                                        name="bsum_ps")
                nc.tensor.matmul(bsum_ps[:, 0:NSB], onesr_sb[:], sums[:],
                                 start=True, stop=True)
                bsum = sm_pool.tile([128, NSB], F32)
                nc.scalar.activation(bsum[:], bsum_ps[:, 0:NSB], AF.Identity)
                for k in range(NCORES):
                    rdests = [None] * NCORES
                    rdests[k] = (0, k)
                    nc.gpsimd.remote_dma_broadcast(
                        zsg[:, k, :], bsum[:], ag_rsem, ag_lsem,
                        rdests=rdests)
                # signals_writable: the trigger "writes" zsg in Tile's view,
                # ordering the copy below after it. The actual
                # wait-for-remote-writes (ag_rsem >= 16) is attached to the
                # copy AFTER TileContext exits — the Tile scheduler's
                # single-core dry-run would deadlock on a sem only remote
                # peers increment.
                nc.gpsimd.trigger_dma(count=None, signals_writable=[zsg[:]])
                zrow = sm_pool.tile([1, NCORES * NSB], F32)
                ag_copy = nc.gpsimd.tensor_copy(zrow[:], zsg[0:1, :, :])
                zg = sm_pool.tile([1, 1], F32)
                nc.vector.reduce_sum(zg[:], zrow[:],
                                     axis=mybir.AxisListType.X)
            else:
                ag_in = dram_pool.tile([1, NSB], F32)
                nc.gpsimd.dma_start(ag_in[:], sums[:])
                ag_out = dram_pool.tile([1, NCORES * NSB], F32)
                nc.gpsimd.collective_compute(
                    "AllGather",
                    mybir.AluOpType.bypass,
                    replica_groups=[list(range(NCORES))],
                    ins=[ag_in.opt()],
                    outs=[ag_out.opt()],
                )
                zs = sm_pool.tile([1, NCORES * NSB], F32)
                nc.gpsimd.dma_start(zs[:], ag_out[:])
                zg = sm_pool.tile([1, 1], F32)
                nc.vector.reduce_sum(zg[:], zs[:], axis=mybir.AxisListType.X)
            invz = sm_pool.tile([1, 1], F32)
            nc.vector.reciprocal(invz[:], zg[:])
            outv = sm_pool.tile([1, SL], F32)
            # scale in quarter chunks alternating ACT/DVE, shipping each
            # chunk as soon as it's done (the row lives on one partition,
            # so these ops are serial per-element — chunking cuts latency)
            ql = SL // 4
            for q in range(4):
                s = slice(q * ql, (q + 1) * ql)
                if q % 2 == 0:
                    nc.scalar.activation(outv[:, s], exps[:, s], AF.Identity,
                                         scale=invz[:])
                else:
                    nc.vector.tensor_scalar_mul(outv[:, s], exps[:, s],
                                                invz[:])
                (nc.sync if q % 2 == 0 else nc.gpsimd).dma_start(
                    out[:, s], outv[:, s])

    if not single_core and ag == "rdma" and not mm_only:
        ag_copy.wait_op(ag_rsem, 16, "sem-ge", check=False)
    _split_multi_waits(nc)
    return nc


def prepare_in_maps(hidden, encoder_output, attn_w, attn_b, v_w):
    import ml_dtypes
    bf16 = ml_dtypes.bfloat16
    hidden = np.asarray(hidden, dtype=np.float32)
    enc = np.asarray(encoder_output, dtype=np.float32)
    attn_w = np.asarray(attn_w, dtype=np.float32)
    attn_b = np.asarray(attn_b, dtype=np.float32)
    v_w = np.asarray(v_w, dtype=np.float32)

    w2t = np.ascontiguousarray(attn_w[:, 2 * H:].T).astype(bf16)   # [H, H]
    w1t = np.ascontiguousarray(attn_w[:, :2 * H].T).astype(bf16)
    hidT = np.ascontiguousarray(hidden.reshape(16, 128).T).astype(bf16)
    bT = np.ascontiguousarray(attn_b.reshape(JC, 128).T)           # [128, 8]
    vwc = np.ascontiguousarray(v_w.reshape(JC, 128).T)             # [128, 8]

    in_maps = []
    for c in range(NCORES):
        encT = np.ascontiguousarray(
            enc[c * SL:(c + 1) * SL, :].T).astype(bf16)
        in_maps.append({
            "encT": encT, "w2t": w2t, "w1s": w1t, "hidT": hidT,
            "bT": bT, "vwc": vwc,
            "onesd": np.ones((128, 1), dtype=np.float32),
            "onesr": np.ones((1, 128), dtype=np.float32),
        })
    return in_maps


_NC_CACHE = {}


def _get_nc(repeat: int = 1):
    if repeat not in _NC_CACHE:
        _NC_CACHE[repeat] = build(repeat)
    return _NC_CACHE[repeat]


def kernel(hidden, encoder_output, attn_w, attn_b, v_w):
    nc = _get_nc(1)
    in_maps = prepare_in_maps(hidden, encoder_output, attn_w, attn_b, v_w)
    res = run_bass_kernel_spmd(nc, in_maps, list(range(NCORES)))
    return np.concatenate([res.results[c]["out"][0] for c in range(NCORES)])
